# revision 91
# baseline (speedup 1.0000x reference)
"""Trainium2 Bass kernel for ModalityAttention (B=4, S=1024, D=2048, H=16, HD=128, RD=64).

Sharding: 8 cores = 4 batches x 2 head-groups (8 heads each).
Each core computes, for its (batch b, head-group g):
  layernorm(x[b]) -> modulation (scale/bias precomputed on host from mod@mod_w)
  -> qkv projection for its 8 heads -> rmsnorm + rope -> attention
  -> partial out-projection (transposed layout) with gate folded in.
Host gathers: out[b] = (partial_g0 + partial_g1).T + x[b]
(residual added on host; vb = out_b*gate folded into the g0 partial on device).

All matmuls run in bf16 (1 cycle/row on the PE vs 4 for fp32; weights cast on
host); stats and softmax sums stay fp32.  Key structure:
- The layernorm + modulation are folded out of the device inner loop entirely:
  W' = (1+scale)*W on the host, plus two "augmented" contraction rows per qkv
  psum group (lhsT rows [1/rstd, -mu] x rhs rows [bias@W, colsum(W')]), with
  the remaining rstd factor applied only at the v evacuation (it cancels
  through rmsnorm for q/k).  xnT is then just raw x^T, produced by DMA-xbar
  transposes straight from DRAM (no PE/PSUM involvement).
- The qkv projection iterates s-tiles in the outer loop against resident
  [128, KT, 512] weight slabs (double-buffered), so it starts ~10us into the
  kernel and overlaps all of phase A; rms/rope and the q/k xbar transposes
  overlap the v projection.
- DMAs are consolidated (HWDGE issue is ~0.6us each, and each issuing queue
  only allows 2 DMAs in flight): weight slabs, whole-wo slab, batched cos/sin
  and output stores; the staged x loads for the stats run on the Pool/SWDGE
  queue so the SP queue's two slots feed the critical xbar transposes.
- PSUM evacuations alternate ACT/DVE (Pool cannot read PSUM); the attention
  exp-sum accumulation runs on DVE; softmax normalization uses a Pool
  partition_all_reduce.
"""
import os, sys

for _p in ("/opt/trn_rl_repo", "/root/.axon_site/_ro/trn_rl_repo", "/root/.axon_site"):
    if os.path.isdir(_p) and _p not in sys.path:
        sys.path.insert(0, _p)

import numpy as np
import concourse.bass as bass
import concourse.bacc as bacc
import concourse.mybir as mybir
import concourse.tile as tile
from concourse import bass_isa
from concourse.masks import make_identity
from concourse.bass_utils import run_bass_kernel_spmd

F32 = mybir.dt.float32
BF16 = mybir.dt.bfloat16
AF = mybir.ActivationFunctionType
S, D, HG, HD, RD = 1024, 2048, 8, 128, 64
NT = S // 128        # 8 s-tiles
KT = D // 128        # 16 d-tiles
GCOLS = HG * HD      # 1024 columns per group per projection
EPS = 1e-6
N_CORES = 8


def _bcast_from_dram(ap, parts, reps=None):
    """DRAM AP -> partition-broadcast (and optional middle-dim repeat) source AP."""
    newap = [[0, parts]]
    if reps is not None:
        newap.append([0, reps])
    newap += list(ap.ap)
    return bass.AP(tensor=ap.tensor, offset=ap.offset, ap=newap)


def build_nc(has_qkv_bias: bool, has_norm_w: bool):
    nc = bacc.Bacc("TRN2", target_bir_lowering=False, debug=False,
                   enable_asserts=True, num_devices=N_CORES)

    x = nc.dram_tensor("x", [S, D], BF16, kind="ExternalInput").ap()
    # second copy of x: the xbar-transpose DMAs read this one, so the tile
    # framework doesn't serialize them against the staged x loads (DRAM
    # dependency tracking is whole-tensor)
    xt_src = nc.dram_tensor("xt_src", [S, D], BF16, kind="ExternalInput").ap()
    cos = nc.dram_tensor("cos", [S, RD // 2], BF16, kind="ExternalInput").ap()
    sin = nc.dram_tensor("sin", [S, RD // 2], BF16, kind="ExternalInput").ap()
    # wq/wk/wv already carry the modulation fold: W' = (1+scale) * W
    wq = nc.dram_tensor("wq", [D, GCOLS], BF16, kind="ExternalInput").ap()
    wk = nc.dram_tensor("wk", [D, GCOLS], BF16, kind="ExternalInput").ap()
    wv = nc.dram_tensor("wv", [D, GCOLS], BF16, kind="ExternalInput").ap()
    wo = nc.dram_tensor("wo", [GCOLS, D], BF16, kind="ExternalInput").ap()
    # augmented contraction rows: row0 = bias@W (pairs with 1/rstd), row1 =
    # colsum(W') (pairs with -mu); columns ordered (proj, n-half, 512)
    waug = nc.dram_tensor("waug", [2, 6 * 512], BF16, kind="ExternalInput").ap()
    # gate / out-bias*gate vectors, host-reshaped to [128, KT]
    gate = nc.dram_tensor("gate", [128, KT], F32, kind="ExternalInput").ap()
    vb = nc.dram_tensor("vb", [128, KT], F32, kind="ExternalInput").ap()
    if has_qkv_bias:
        bq = nc.dram_tensor("bq", [GCOLS], BF16, kind="ExternalInput").ap()
        bk = nc.dram_tensor("bk", [GCOLS], BF16, kind="ExternalInput").ap()
        bv = nc.dram_tensor("bv", [GCOLS], BF16, kind="ExternalInput").ap()
    if has_norm_w:
        wqn = nc.dram_tensor("wqn", [HD], BF16, kind="ExternalInput").ap()
        wkn = nc.dram_tensor("wkn", [HD], BF16, kind="ExternalInput").ap()
    out_t = nc.dram_tensor("out_t", [D, S], F32, kind="ExternalOutput").ap()

    with tile.TileContext(nc) as tc:
        # ======== LEFT stack bottom: small persistent constants ====================
        misc_cm = tc.tile_pool(name="misc", bufs=1, side="left")
        misc = misc_cm.__enter__()
        ident = misc.tile([128, 128], BF16)
        make_identity(nc, ident)
        eps_t = misc.tile([128, 1], F32)
        nc.vector.memset(eps_t, EPS)
        eps128_t = misc.tile([128, 1], F32)
        nc.vector.memset(eps128_t, HD * EPS)
        gate_sb = misc.tile([128, KT], F32)
        vb_sb = misc.tile([128, KT], F32)
        rrk_all = misc.tile([128, NT, HG], F32)   # scaled k-rms reciprocals
        rstd_all = misc.tile([128, NT], F32)      # layernorm 1/std per s-tile
        augT = misc.tile([2, S], BF16)            # rows [1/rstd, -mu] per s
        packs = misc.tile([128, 2, NT], BF16)     # pre-transpose aug values
        waug_sb = misc.tile([2, 6 * 512], BF16)
        if has_norm_w:
            wqn_b = misc.tile([128, HG, HD], BF16)
            wkn_b = misc.tile([128, HG, HD], BF16)
        cs_c = misc.tile([128, NT, RD // 2], BF16)   # cos, s-tile m in dim 1
        cs_s = misc.tile([128, NT, RD // 2], BF16)

        # ======== RIGHT stack: wo slab (whole kernel) + natural qkv ===============
        wop_cm = tc.tile_pool(name="wopool", bufs=1, side="right")
        wop = wop_cm.__enter__()
        wo_sb = wop.tile([128, HG, D], BF16)   # wo[kb*128+p, d]
        v_cm = tc.tile_pool(name="vpool", bufs=1, side="right")
        v_p = v_cm.__enter__()
        vnat = v_p.tile([128, NT, GCOLS], BF16)
        natqk_cm = tc.tile_pool(name="natqk", bufs=1, side="right")
        natqk = natqk_cm.__enter__()
        qnat = natqk.tile([128, NT, GCOLS], BF16)
        knat = natqk.tile([128, NT, GCOLS], BF16)
        w_cm = tc.tile_pool(name="wstream", bufs=2, side="right")
        w_p = w_cm.__enter__()

        # weight slabs for the qkv projections: 6 groups of [128, KT, 512]
        wslabs = {}

        def emit_wslab(g, eng=None):
            proj, n = divmod(g, 2)
            wdram = (wq, wk, wv)[proj]
            t = w_p.tile([128, KT, 512], BF16, tag="wslab")
            src = wdram.rearrange("(kb p) c -> p kb c", p=128)[
                :, :, n * 512:(n + 1) * 512]
            (eng or nc.sync).dma_start(out=t, in_=src)
            wslabs[g] = t

        # ======== phase A: layernorm + modulation + transpose -> xnT ===============
        xnT_cm = tc.tile_pool(name="xnT", bufs=1, side="left")
        xnT_p = xnT_cm.__enter__()
        xnT = xnT_p.tile([128, KT, S], BF16)  # [d_in_tile, d_tile, s]

        avec_cm = tc.tile_pool(name="phA_vec", bufs=1, side="left")
        avec = avec_cm.__enter__()
        if has_qkv_bias:
            bq_b = avec.tile([128, GCOLS], BF16)
            nc.sync.dma_start(out=bq_b, in_=_bcast_from_dram(bq, 128))
            bk_b = avec.tile([128, GCOLS], BF16)
            nc.sync.dma_start(out=bk_b, in_=_bcast_from_dram(bk, 128))
            bv_b = avec.tile([128, GCOLS], BF16)
            nc.sync.dma_start(out=bv_b, in_=_bcast_from_dram(bv, 128))

        a_cm = tc.tile_pool(name="phA", bufs=4, side="left")
        a_p = a_cm.__enter__()
        a_small_cm = tc.tile_pool(name="phA_small", bufs=4, side="left")
        a_small = a_small_cm.__enter__()
        pst_cm = tc.tile_pool(name="ps_tr", bufs=4, space="PSUM")
        pst = pst_cm.__enter__()

        # first weight slab via the Pool/SWDGE channel, issued before anything
        # else: its transfer overlaps the first xbar transposes instead of
        # queueing behind them in SP's two in-flight slots
        emit_wslab(0, eng=nc.gpsimd)
        for i in range(NT):
            # raw x^T straight into xnT via the DMA xbar, READ FROM DRAM so it
            # has no dependency on (and imposes none on) the staged x tiles —
            # the SP queue streams loads + transposes with no semaphore waits.
            nc.sync.dma_start_transpose(
                out=xnT[:, :, i * 128:(i + 1) * 128],
                in_=xt_src[i * 128:(i + 1) * 128, :])
            if i == 4:
                emit_wslab(1)
            # staged x load for the layernorm stats, via SWDGE on the idle
            # Pool queue — each HWDGE queue only allows 2 DMAs in flight, and
            # SP's slots are needed for the xnT transposes + weight slabs
            xb = a_p.tile([128, D], BF16, tag="xb")
            nc.gpsimd.dma_start(out=xb, in_=x[i * 128:(i + 1) * 128, :])
            if i == 0:
                nc.sync.dma_start(out=waug_sb, in_=waug)
            stats = a_small.tile([128, 4, 6], F32, tag="stats")
            xv = xb.rearrange("p (c f) -> p c f", c=4)
            for c in range(4):
                nc.vector.bn_stats(out=stats[:, c, :], in_=xv[:, c, :])
            mv = a_small.tile([128, 2], F32, tag="mv")
            nc.vector.bn_aggr(out=mv, in_=stats)
            # pack [1/rstd, -mu]; the transpose to augT rows happens inside
            # phase B's first group so it never gates the B matmul stream
            nc.scalar.activation(out=packs[:, 0, i:i + 1], in_=mv[:, 1:2],
                                 func=AF.Sqrt, bias=eps_t, scale=1.0)
            nc.vector.reciprocal(out=rstd_all[:, i:i + 1], in_=packs[:, 0, i:i + 1])
            nc.scalar.mul(out=packs[:, 1, i:i + 1], in_=mv[:, 0:1], mul=-1.0)

        # deferred misc loads (consumed in phases C/E/F) + wo slab prefetch
        nc.sync.dma_start(out=gate_sb, in_=gate)
        nc.sync.dma_start(out=vb_sb, in_=vb)
        if has_norm_w:
            nc.sync.dma_start(out=wqn_b, in_=_bcast_from_dram(wqn, 128, reps=HG))
            nc.sync.dma_start(out=wkn_b, in_=_bcast_from_dram(wkn, 128, reps=HG))
        nc.sync.dma_start(out=cs_c, in_=cos.rearrange("(m p) c -> p m c", p=128))
        nc.sync.dma_start(out=cs_s, in_=sin.rearrange("(m p) c -> p m c", p=128))
        # queue the remaining weight slabs now: each waits on its ping-pong
        # buffer only, so it streams in just ahead of its use
        for _g in range(2, 6):
            emit_wslab(_g)

        a_small_cm.__exit__(None, None, None)
        a_cm.__exit__(None, None, None)

        # phase C/D pools opened BEFORE phase B emission so the rms/rope work and
        # the q/k transposes can overlap the tail of the qkv matmuls.
        c_cm = tc.tile_pool(name="phC", bufs=2, side="left")
        c_p = c_cm.__enter__()
        c_small_cm = tc.tile_pool(name="phC_small", bufs=2, side="left")
        c_small = c_small_cm.__enter__()
        qkT_cm = tc.tile_pool(name="qkT", bufs=1, side="left")
        qkT_p = qkT_cm.__enter__()
        qT = qkT_p.tile([128, HG, S], BF16)
        kT = qkT_p.tile([128, HG, S], BF16)

        # ======== phase B: qkv projections (natural layout, m-outer) ==============
        psb_cm = tc.tile_pool(name="ps_qkv", bufs=4, space="PSUM")
        psb = psb_cm.__enter__()

        for g in range(6):
            proj, n = divmod(g, 2)
            nat = (qnat, knat, vnat)[proj]
            if g not in wslabs:
                emit_wslab(g)
            wslab = wslabs.pop(g)
            for m in range(NT):
                if g == 0 and m > 0:
                    # build the two aug lhsT rows for this s-tile: a tiny PE
                    # transpose interleaved with the matmul stream (its stats
                    # are ready well before PE reaches this point)
                    ptA = pst.tile([2, 128], BF16, tag="ptA")
                    nc.tensor.transpose(ptA, packs[:, :, m], ident)
                    nc.scalar.copy(out=augT[:, m * 128:(m + 1) * 128], in_=ptA)
                ps = psb.tile([128, 512], F32, tag="psb")
                for k in range(KT):
                    nc.tensor.matmul(ps, xnT[:, k, m * 128:(m + 1) * 128],
                                     wslab[:, k, :],
                                     start=(k == 0), stop=False)
                if g == 0 and m == 0:
                    # m=0's aug rows go AFTER its k-loop: its stats chain is
                    # the longest pole at startup and PE is strictly in-order
                    ptA = pst.tile([2, 128], BF16, tag="ptA")
                    nc.tensor.transpose(ptA, packs[:, :, m], ident)
                    nc.scalar.copy(out=augT[:, m * 128:(m + 1) * 128], in_=ptA)
                # augmented rows: += (1/rstd)*b' + (-mu)*colsum(W')
                nc.tensor.matmul(ps, augT[:, m * 128:(m + 1) * 128],
                                 waug_sb[:, g * 512:(g + 1) * 512],
                                 start=False, stop=True)
                dst = nat[:, m, n * 512:(n + 1) * 512]
                if proj == 2:
                    # v needs the layernorm 1/std (cancels via rmsnorm for q/k)
                    if m % 2 == 0:
                        nc.scalar.activation(out=dst, in_=ps, func=AF.Identity,
                                             scale=rstd_all[:, m:m + 1])
                    else:
                        nc.vector.tensor_scalar(
                            out=dst, in0=ps, scalar1=rstd_all[:, m:m + 1],
                            scalar2=None, op0=mybir.AluOpType.mult)
                elif (g * NT + m) % 2 == 0:
                    nc.scalar.copy(out=dst, in_=ps)
                else:
                    nc.vector.tensor_copy(out=dst, in_=ps)
        if has_qkv_bias:
            for m in range(NT):
                nc.gpsimd.tensor_add(out=qnat[:, m, :], in0=qnat[:, m, :], in1=bq_b)
                nc.gpsimd.tensor_add(out=knat[:, m, :], in0=knat[:, m, :], in1=bk_b)
                nc.gpsimd.tensor_add(out=vnat[:, m, :], in0=vnat[:, m, :], in1=bv_b)

        psb_cm.__exit__(None, None, None)
        pst_cm.__exit__(None, None, None)
        w_cm.__exit__(None, None, None)

        # ======== phase C+D: rmsnorm + rope, then per-head transposes =============
        for m in range(NT):
            qm = qnat[:, m, :]
            km = knat[:, m, :]
            ct = cs_c[:, m, :]
            st = cs_s[:, m, :]
            cb = ct.unsqueeze(1).broadcast_to([128, HG, RD // 2])
            sb_ = st.unsqueeze(1).broadcast_to([128, HG, RD // 2])

            # rms stats (on raw q/k, before norm-w and rope)
            sq = c_p.tile([128, GCOLS], BF16, tag="sqk")
            nc.vector.tensor_mul(out=sq, in0=qm, in1=qm)
            ssq = c_small.tile([128, HG], F32, tag="ssq")
            nc.vector.reduce_sum(out=ssq, in_=sq.rearrange("p (h d) -> p h d", h=HG),
                                 axis=mybir.AxisListType.X)
            rrq = c_small.tile([128, HG], F32, tag="rrq")
            nc.scalar.activation(out=rrq, in_=ssq, func=AF.Sqrt,
                                 bias=eps_t, scale=1.0 / HD)
            nc.vector.reciprocal(out=rrq, in_=rrq)

            sk_ = c_p.tile([128, GCOLS], BF16, tag="sqk")
            nc.vector.tensor_mul(out=sk_, in0=km, in1=km)
            ssk = c_small.tile([128, HG], F32, tag="ssk")
            nc.vector.reduce_sum(out=ssk, in_=sk_.rearrange("p (h d) -> p h d", h=HG),
                                 axis=mybir.AxisListType.X)
            nc.scalar.activation(out=rrk_all[:, m, :], in_=ssk, func=AF.Sqrt,
                                 bias=eps128_t, scale=1.0)
            nc.vector.reciprocal(out=rrk_all[:, m, :], in_=rrk_all[:, m, :])

            if has_norm_w:
                nc.vector.tensor_mul(out=qm.rearrange("p (h d) -> p h d", h=HG),
                                     in0=qm.rearrange("p (h d) -> p h d", h=HG),
                                     in1=wqn_b)
                nc.vector.tensor_mul(out=km.rearrange("p (h d) -> p h d", h=HG),
                                     in0=km.rearrange("p (h d) -> p h d", h=HG),
                                     in1=wkn_b)

            for mm in (qm, km):
                mv_ = mm.rearrange("p (h i two) -> p h i two", h=HG, two=2)
                x0 = mv_[:, :, 0:RD // 2, 0]
                x1 = mv_[:, :, 0:RD // 2, 1]
                t0 = c_small.tile([128, HG, RD // 2], BF16, tag="t0")
                t1 = c_small.tile([128, HG, RD // 2], BF16, tag="t1")
                t2 = c_small.tile([128, HG, RD // 2], BF16, tag="t2")
                t3 = c_small.tile([128, HG, RD // 2], BF16, tag="t3")
                nc.vector.tensor_mul(out=t0, in0=x0, in1=cb)
                nc.vector.tensor_mul(out=t1, in0=x1, in1=sb_)
                nc.vector.tensor_mul(out=t2, in0=x0, in1=sb_)
                nc.vector.tensor_mul(out=t3, in0=x1, in1=cb)
                nc.gpsimd.tensor_sub(out=x0, in0=t0, in1=t1)
                nc.gpsimd.tensor_add(out=x1, in0=t2, in1=t3)

            # apply q rms reciprocal (k's is folded into the exp scale later)
            rrq_b = rrq.unsqueeze(2).broadcast_to([128, HG, HD])
            nc.vector.tensor_mul(out=qm.rearrange("p (h d) -> p h d", h=HG),
                                 in0=qm.rearrange("p (h d) -> p h d", h=HG),
                                 in1=rrq_b)

            # ---- phase D fused: transpose q, k of this s-tile -> [hd, head, s]
            # on the DMA xbar (PE and ACT/DVE stay free for qkv/attention
            # work).  Issued from the ACT queue — the SP queue is busy waiting
            # out weight-slab buffer reuse at this point.
            nc.scalar.dma_start_transpose(
                out=qT[:, :, m * 128:(m + 1) * 128], in_=qnat[:, m, :])
            nc.scalar.dma_start_transpose(
                out=kT[:, :, m * 128:(m + 1) * 128], in_=knat[:, m, :])

        # wo slab load deferred to here: DMA engines are busy with x/xnT/slab
        # traffic early on, and wo isn't consumed until the out projection
        nc.sync.dma_start(out=wo_sb, in_=wo.rearrange("(kb p) d -> p kb d", p=128))

        natqk_cm.__exit__(None, None, None)

        # ======== phase E: attention per head ======================================
        oT_cm = tc.tile_pool(name="oT", bufs=1, side="left")
        oT_p = oT_cm.__enter__()
        oT = oT_p.tile([128, HG, S], BF16)
        at_cm = tc.tile_pool(name="attn", bufs=6, side="left")
        at_p = at_cm.__enter__()
        rs_cm = tc.tile_pool(name="rsb", bufs=3, side="left")
        rs_p = rs_cm.__enter__()
        pssc_cm = tc.tile_pool(name="ps_sc", bufs=2, space="PSUM")
        pssc = pssc_cm.__enter__()
        pso_cm = tc.tile_pool(name="ps_o", bufs=2, space="PSUM")
        pso = pso_cm.__enter__()

        for h in range(HG):
            o_ps = pso.tile([128, S], F32, tag="o_ps")
            acc = rs_p.tile([128, S], BF16, tag="acc")
            for m in range(NT):
                sc = pssc.tile([128, S], F32, tag="sc")
                lhs_k = kT[:, h, m * 128:(m + 1) * 128]
                nc.tensor.matmul(sc[:, 0:512], lhs_k, qT[:, h, 0:512],
                                 start=True, stop=True)
                nc.tensor.matmul(sc[:, 512:1024], lhs_k, qT[:, h, 512:1024],
                                 start=True, stop=True)
                at = at_p.tile([128, S], BF16, tag="at", name="at")
                nc.scalar.activation(out=at, in_=sc, func=AF.Exp,
                                     scale=rrk_all[:, m, h:h + 1])
                # accumulate exp tiles on DVE (sums over the m-tiles)
                if m == 0:
                    nc.vector.tensor_copy(out=acc, in_=at)
                else:
                    nc.vector.tensor_add(out=acc, in0=acc, in1=at)
                first, last = (m == 0), (m == NT - 1)
                v_mh = vnat[:, m, h * 128:(h + 1) * 128]
                nc.tensor.matmul(o_ps[:, 0:512], v_mh, at[:, 0:512],
                                 start=first, stop=last)
                nc.tensor.matmul(o_ps[:, 512:1024], v_mh, at[:, 512:1024],
                                 start=first, stop=last)
            # sum over the sk partitions -> broadcast row, then normalize.
            # Done in column halves: halves the latency of the
            # all_reduce->reciprocal->mul chain, which otherwise stalls the
            # out-projection's first psum group waiting on the last head.
            sums_b = rs_p.tile([128, S], F32, tag="sums_b")
            for hh in range(2):
                cl = slice(hh * 512, (hh + 1) * 512)
                nc.gpsimd.partition_all_reduce(sums_b[:, cl], acc[:, cl], 128,
                                               bass_isa.ReduceOp.add)
                nc.vector.reciprocal(out=sums_b[:, cl], in_=sums_b[:, cl])
                nc.vector.tensor_mul(out=oT[:, h, cl], in0=o_ps[:, cl],
                                     in1=sums_b[:, cl])

        pso_cm.__exit__(None, None, None)
        pssc_cm.__exit__(None, None, None)
        rs_cm.__exit__(None, None, None)
        at_cm.__exit__(None, None, None)
        v_cm.__exit__(None, None, None)

        # ======== phase F: out projection (transposed out) =========================
        f_cm = tc.tile_pool(name="phF", bufs=2, side="left")
        f_p = f_cm.__enter__()
        psf_cm = tc.tile_pool(name="ps_out", bufs=2, space="PSUM")
        psf = psf_cm.__enter__()
        for mb in range(KT // 2):
            stage = f_p.tile([128, 2, S], F32, tag="stage")
            for mm in range(2):
                m = mb * 2 + mm
                po = psf.tile([128, S], F32, tag="po")
                for kb in range(HG):
                    first, last = (kb == 0), (kb == HG - 1)
                    nc.tensor.matmul(po[:, 0:512],
                                     wo_sb[:, kb, m * 128:(m + 1) * 128],
                                     oT[:, kb, 0:512], start=first, stop=last)
                    nc.tensor.matmul(po[:, 512:1024],
                                     wo_sb[:, kb, m * 128:(m + 1) * 128],
                                     oT[:, kb, 512:1024], start=first, stop=last)
                nc.scalar.activation(out=stage[:, mm, :], in_=po, func=AF.Identity,
                                     bias=vb_sb[:, m:m + 1], scale=gate_sb[:, m:m + 1])
            nc.sync.dma_start(
                out=out_t[mb * 256:(mb + 1) * 256, :].rearrange(
                    "(mm p) s -> p mm s", p=128),
                in_=stage)
        psf_cm.__exit__(None, None, None)
        f_cm.__exit__(None, None, None)
        oT_cm.__exit__(None, None, None)
        qkT_cm.__exit__(None, None, None)
        c_small_cm.__exit__(None, None, None)
        c_cm.__exit__(None, None, None)
        avec_cm.__exit__(None, None, None)
        xnT_cm.__exit__(None, None, None)
        wop_cm.__exit__(None, None, None)
        misc_cm.__exit__(None, None, None)

    nc.compile()
    return nc


_NC_CACHE = {}


def _get_nc(has_qkv_bias, has_norm_w):
    key = (has_qkv_bias, has_norm_w)
    if key not in _NC_CACHE:
        _NC_CACHE[key] = build_nc(*key)
    return _NC_CACHE[key]


def prep_in_maps(x, mod, cos, sin, qkv_w, qkv_b, mod_w, mod_b, out_w, out_b,
                 norm_q_w, norm_k_w):
    """Host-side sharding. Returns (in_maps, flags, x_np)."""
    x = np.asarray(x, dtype=np.float32)
    m3 = np.asarray(mod, np.float32) @ np.asarray(mod_w, np.float32) \
        + np.asarray(mod_b, np.float32)
    bias, scale, gatef = np.split(m3, 3, axis=-1)          # [B, D] each
    scale1p = (1.0 + scale).astype(np.float32)
    vbf = (np.asarray(out_b, np.float32)[None, :] * gatef).astype(np.float32)

    qkv_b = np.asarray(qkv_b, np.float32)
    has_qkv_bias = bool(np.any(qkv_b != 0.0))
    has_norm_w = not (np.allclose(norm_q_w, 1.0) and np.allclose(norm_k_w, 1.0))

    import ml_dtypes
    bf16 = ml_dtypes.bfloat16
    cosc = np.ascontiguousarray(np.asarray(cos, np.float32).astype(bf16))
    sinc = np.ascontiguousarray(np.asarray(sin, np.float32).astype(bf16))
    qkv_w = np.asarray(qkv_w, np.float32)
    out_w = np.asarray(out_w, np.float32).astype(bf16)
    # b' = bias @ W (the modulation bias term, unscaled W): [B, 3D]
    bprime = bias @ qkv_w

    in_maps = []
    for c in range(N_CORES):
        b, g = divmod(c, 2)
        lo = g * GCOLS
        # fold (1+scale) into the weight rows for this batch
        wmod = qkv_w * scale1p[b][:, None]          # [D, 3D] fp32
        cols = [wmod[:, p * 2048 + lo:p * 2048 + lo + GCOLS] for p in range(3)]
        bcols = [bprime[b, p * 2048 + lo:p * 2048 + lo + GCOLS] for p in range(3)]
        w1 = np.concatenate([cw.sum(axis=0) for cw in cols])       # [3*GCOLS]
        b2 = np.concatenate(bcols)                                 # [3*GCOLS]
        xbf = np.ascontiguousarray(x[b].astype(bf16))
        im = {
            "x": xbf,
            "xt_src": xbf.copy(),
            "cos": cosc, "sin": sinc,
            "wq": np.ascontiguousarray(cols[0].astype(bf16)),
            "wk": np.ascontiguousarray(cols[1].astype(bf16)),
            "wv": np.ascontiguousarray(cols[2].astype(bf16)),
            "wo": np.ascontiguousarray(out_w[lo:lo + GCOLS, :]),
            "waug": np.ascontiguousarray(
                np.stack([b2, w1]).astype(bf16)),                  # [2, 3*GCOLS]
            "gate": np.ascontiguousarray(gatef[b].reshape(KT, 128).T),
            "vb": np.ascontiguousarray(
                (vbf[b] if g == 0 else np.zeros_like(vbf[b])).reshape(KT, 128).T),
        }
        if has_qkv_bias:
            im["bq"] = np.ascontiguousarray(qkv_b[lo:lo + GCOLS].astype(bf16))
            im["bk"] = np.ascontiguousarray(
                qkv_b[2048 + lo:2048 + lo + GCOLS].astype(bf16))
            im["bv"] = np.ascontiguousarray(
                qkv_b[4096 + lo:4096 + lo + GCOLS].astype(bf16))
        if has_norm_w:
            im["wqn"] = np.ascontiguousarray(
                np.asarray(norm_q_w, np.float32).astype(bf16))
            im["wkn"] = np.ascontiguousarray(
                np.asarray(norm_k_w, np.float32).astype(bf16))
        in_maps.append(im)
    return in_maps, (has_qkv_bias, has_norm_w), x


def gather(results, x):
    B = x.shape[0]
    outs = []
    for b in range(B):
        p = results[2 * b]["out_t"] + results[2 * b + 1]["out_t"]   # [D, S]
        outs.append(p.T + x[b])
    return np.stack(outs).astype(np.float32)


def kernel(**inputs) -> np.ndarray:
    in_maps, flags, x = prep_in_maps(**inputs)
    nc = _get_nc(*flags)
    res = run_bass_kernel_spmd(nc, in_maps, core_ids=list(range(N_CORES)))
    return gather(res.results, x)


if __name__ == "__main__":
    import time
    t0 = time.time()
    nc = build_nc(False, False)
    print("build+compile ok in", time.time() - t0, "s")


# revision 92
# speedup vs baseline: 1.0143x; 1.0143x over previous
"""Trainium2 Bass kernel for ModalityAttention (B=4, S=1024, D=2048, H=16, HD=128, RD=64).

Sharding: 8 cores = 4 batches x 2 head-groups (8 heads each).
Each core computes, for its (batch b, head-group g):
  layernorm(x[b]) -> modulation (scale/bias precomputed on host from mod@mod_w)
  -> qkv projection for its 8 heads -> rmsnorm + rope -> attention
  -> partial out-projection (transposed layout) with gate folded in.
Host gathers: out[b] = (partial_g0 + partial_g1).T + x[b]
(residual added on host; vb = out_b*gate folded into the g0 partial on device).

All matmuls run in bf16 (1 cycle/row on the PE vs 4 for fp32; weights cast on
host); stats and softmax sums stay fp32.  Key structure:
- The layernorm + modulation are folded out of the device inner loop entirely:
  W' = (1+scale)*W on the host, plus two "augmented" contraction rows per qkv
  psum group (lhsT rows [1/rstd, -mu] x rhs rows [bias@W, colsum(W')]), with
  the remaining rstd factor applied only at the v evacuation (it cancels
  through rmsnorm for q/k).  xnT is then just raw x^T, produced by DMA-xbar
  transposes straight from DRAM (no PE/PSUM involvement).
- The qkv projection iterates s-tiles in the outer loop against resident
  [128, KT, 512] weight slabs (double-buffered), so it starts ~10us into the
  kernel and overlaps all of phase A; rms/rope and the q/k xbar transposes
  overlap the v projection.
- DMAs are consolidated (HWDGE issue is ~0.6us each, and each issuing queue
  only allows 2 DMAs in flight): weight slabs, whole-wo slab, batched cos/sin
  and output stores; the staged x loads for the stats run on the Pool/SWDGE
  queue so the SP queue's two slots feed the critical xbar transposes.
- PSUM evacuations alternate ACT/DVE (Pool cannot read PSUM); the attention
  exp-sum accumulation runs on DVE; softmax normalization uses a Pool
  partition_all_reduce.
"""
import os, sys

for _p in ("/opt/trn_rl_repo", "/root/.axon_site/_ro/trn_rl_repo", "/root/.axon_site"):
    if os.path.isdir(_p) and _p not in sys.path:
        sys.path.insert(0, _p)

import numpy as np
import concourse.bass as bass
import concourse.bacc as bacc
import concourse.mybir as mybir
import concourse.tile as tile
from concourse import bass_isa
from concourse.masks import make_identity
from concourse.bass_utils import run_bass_kernel_spmd

F32 = mybir.dt.float32
BF16 = mybir.dt.bfloat16
AF = mybir.ActivationFunctionType
S, D, HG, HD, RD = 1024, 2048, 8, 128, 64
NT = S // 128        # 8 s-tiles
KT = D // 128        # 16 d-tiles
GCOLS = HG * HD      # 1024 columns per group per projection
EPS = 1e-6
N_CORES = 8


def _bcast_from_dram(ap, parts, reps=None):
    """DRAM AP -> partition-broadcast (and optional middle-dim repeat) source AP."""
    newap = [[0, parts]]
    if reps is not None:
        newap.append([0, reps])
    newap += list(ap.ap)
    return bass.AP(tensor=ap.tensor, offset=ap.offset, ap=newap)


def build_nc(has_qkv_bias: bool, has_norm_w: bool):
    nc = bacc.Bacc("TRN2", target_bir_lowering=False, debug=False,
                   enable_asserts=True, num_devices=N_CORES)

    x = nc.dram_tensor("x", [S, D], BF16, kind="ExternalInput").ap()
    # second copy of x: the xbar-transpose DMAs read this one, so the tile
    # framework doesn't serialize them against the staged x loads (DRAM
    # dependency tracking is whole-tensor)
    xt_src = nc.dram_tensor("xt_src", [S, D], BF16, kind="ExternalInput").ap()
    cos = nc.dram_tensor("cos", [S, RD // 2], BF16, kind="ExternalInput").ap()
    sin = nc.dram_tensor("sin", [S, RD // 2], BF16, kind="ExternalInput").ap()
    # wq/wk/wv already carry the modulation fold: W' = (1+scale) * W
    wq = nc.dram_tensor("wq", [D, GCOLS], BF16, kind="ExternalInput").ap()
    wk = nc.dram_tensor("wk", [D, GCOLS], BF16, kind="ExternalInput").ap()
    wv = nc.dram_tensor("wv", [D, GCOLS], BF16, kind="ExternalInput").ap()
    wo = nc.dram_tensor("wo", [GCOLS, D], BF16, kind="ExternalInput").ap()
    # augmented contraction rows: row0 = bias@W (pairs with 1/rstd), row1 =
    # colsum(W') (pairs with -mu); columns ordered (proj, n-half, 512)
    waug = nc.dram_tensor("waug", [2, 6 * 512], BF16, kind="ExternalInput").ap()
    # gate / out-bias*gate vectors, host-reshaped to [128, KT]
    gate = nc.dram_tensor("gate", [128, KT], F32, kind="ExternalInput").ap()
    vb = nc.dram_tensor("vb", [128, KT], F32, kind="ExternalInput").ap()
    if has_qkv_bias:
        bq = nc.dram_tensor("bq", [GCOLS], BF16, kind="ExternalInput").ap()
        bk = nc.dram_tensor("bk", [GCOLS], BF16, kind="ExternalInput").ap()
        bv = nc.dram_tensor("bv", [GCOLS], BF16, kind="ExternalInput").ap()
    if has_norm_w:
        wqn = nc.dram_tensor("wqn", [HD], BF16, kind="ExternalInput").ap()
        wkn = nc.dram_tensor("wkn", [HD], BF16, kind="ExternalInput").ap()
    out_t = nc.dram_tensor("out_t", [D, S], F32, kind="ExternalOutput").ap()

    with tile.TileContext(nc) as tc:
        # ======== LEFT stack bottom: small persistent constants ====================
        misc_cm = tc.tile_pool(name="misc", bufs=1, side="left")
        misc = misc_cm.__enter__()
        ident = misc.tile([128, 128], BF16)
        make_identity(nc, ident)
        eps_t = misc.tile([128, 1], F32)
        nc.vector.memset(eps_t, EPS)
        eps128_t = misc.tile([128, 1], F32)
        nc.vector.memset(eps128_t, HD * EPS)
        gate_sb = misc.tile([128, KT], F32)
        vb_sb = misc.tile([128, KT], F32)
        rrk_all = misc.tile([128, NT, HG], F32)   # scaled k-rms reciprocals
        rstd_all = misc.tile([128, NT], F32)      # layernorm 1/std per s-tile
        augT = misc.tile([2, S], BF16)            # rows [1/rstd, -mu] per s
        packs = misc.tile([128, 2, NT], BF16)     # pre-transpose aug values
        waug_sb = misc.tile([2, 6 * 512], BF16)
        if has_norm_w:
            wqn_b = misc.tile([128, HG, HD], BF16)
            wkn_b = misc.tile([128, HG, HD], BF16)
        cs_c = misc.tile([128, NT, RD // 2], BF16)   # cos, s-tile m in dim 1
        cs_s = misc.tile([128, NT, RD // 2], BF16)

        # ======== RIGHT stack: wo slab (whole kernel) + natural qkv ===============
        wop_cm = tc.tile_pool(name="wopool", bufs=1, side="right")
        wop = wop_cm.__enter__()
        wo_sb = wop.tile([128, HG, D], BF16)   # wo[kb*128+p, d]
        v_cm = tc.tile_pool(name="vpool", bufs=1, side="right")
        v_p = v_cm.__enter__()
        vnat = v_p.tile([128, NT, GCOLS], BF16)
        natqk_cm = tc.tile_pool(name="natqk", bufs=1, side="right")
        natqk = natqk_cm.__enter__()
        qnat = natqk.tile([128, NT, GCOLS], BF16)
        knat = natqk.tile([128, NT, GCOLS], BF16)
        w_cm = tc.tile_pool(name="wstream", bufs=2, side="right")
        w_p = w_cm.__enter__()

        # weight slabs for the qkv projections: 6 groups of [128, KT, 512]
        wslabs = {}

        def emit_wslab(g, eng=None):
            proj, n = divmod(g, 2)
            wdram = (wq, wk, wv)[proj]
            t = w_p.tile([128, KT, 512], BF16, tag="wslab")
            src = wdram.rearrange("(kb p) c -> p kb c", p=128)[
                :, :, n * 512:(n + 1) * 512]
            (eng or nc.sync).dma_start(out=t, in_=src)
            wslabs[g] = t

        # ======== phase A: layernorm + modulation + transpose -> xnT ===============
        xnT_cm = tc.tile_pool(name="xnT", bufs=1, side="left")
        xnT_p = xnT_cm.__enter__()
        xnT = xnT_p.tile([128, KT, S], BF16)  # [d_in_tile, d_tile, s]

        avec_cm = tc.tile_pool(name="phA_vec", bufs=1, side="left")
        avec = avec_cm.__enter__()
        if has_qkv_bias:
            bq_b = avec.tile([128, GCOLS], BF16)
            nc.sync.dma_start(out=bq_b, in_=_bcast_from_dram(bq, 128))
            bk_b = avec.tile([128, GCOLS], BF16)
            nc.sync.dma_start(out=bk_b, in_=_bcast_from_dram(bk, 128))
            bv_b = avec.tile([128, GCOLS], BF16)
            nc.sync.dma_start(out=bv_b, in_=_bcast_from_dram(bv, 128))

        a_cm = tc.tile_pool(name="phA", bufs=4, side="left")
        a_p = a_cm.__enter__()
        a_small_cm = tc.tile_pool(name="phA_small", bufs=4, side="left")
        a_small = a_small_cm.__enter__()
        pst_cm = tc.tile_pool(name="ps_tr", bufs=4, space="PSUM")
        pst = pst_cm.__enter__()

        for i in range(NT):
            # raw x^T straight into xnT via the DMA xbar, READ FROM DRAM so it
            # has no dependency on (and imposes none on) the staged x tiles —
            # the SP queue streams loads + transposes with no semaphore waits.
            nc.sync.dma_start_transpose(
                out=xnT[:, :, i * 128:(i + 1) * 128],
                in_=xt_src[i * 128:(i + 1) * 128, :])
            if i == 1:
                emit_wslab(0)
            if i == 4:
                emit_wslab(1)
            # staged x load for the layernorm stats, via SWDGE on the idle
            # Pool queue — each HWDGE queue only allows 2 DMAs in flight, and
            # SP's slots are needed for the xnT transposes + weight slabs
            xb = a_p.tile([128, D], BF16, tag="xb")
            nc.gpsimd.dma_start(out=xb, in_=x[i * 128:(i + 1) * 128, :])
            if i == 0:
                nc.sync.dma_start(out=waug_sb, in_=waug)
            stats = a_small.tile([128, 4, 6], F32, tag="stats")
            xv = xb.rearrange("p (c f) -> p c f", c=4)
            for c in range(4):
                nc.vector.bn_stats(out=stats[:, c, :], in_=xv[:, c, :])
            mv = a_small.tile([128, 2], F32, tag="mv")
            nc.vector.bn_aggr(out=mv, in_=stats)
            # pack [1/rstd, -mu]; the transpose to augT rows happens inside
            # phase B's first group so it never gates the B matmul stream
            nc.scalar.activation(out=packs[:, 0, i:i + 1], in_=mv[:, 1:2],
                                 func=AF.Sqrt, bias=eps_t, scale=1.0)
            nc.vector.reciprocal(out=rstd_all[:, i:i + 1], in_=packs[:, 0, i:i + 1])
            nc.scalar.mul(out=packs[:, 1, i:i + 1], in_=mv[:, 0:1], mul=-1.0)

        # deferred misc loads (consumed in phases C/E/F) + wo slab prefetch
        nc.sync.dma_start(out=gate_sb, in_=gate)
        nc.sync.dma_start(out=vb_sb, in_=vb)
        if has_norm_w:
            nc.sync.dma_start(out=wqn_b, in_=_bcast_from_dram(wqn, 128, reps=HG))
            nc.sync.dma_start(out=wkn_b, in_=_bcast_from_dram(wkn, 128, reps=HG))
        nc.sync.dma_start(out=cs_c, in_=cos.rearrange("(m p) c -> p m c", p=128))
        nc.sync.dma_start(out=cs_s, in_=sin.rearrange("(m p) c -> p m c", p=128))
        # queue the remaining weight slabs now: each waits on its ping-pong
        # buffer only, so it streams in just ahead of its use
        for _g in range(2, 6):
            emit_wslab(_g)

        a_small_cm.__exit__(None, None, None)
        a_cm.__exit__(None, None, None)

        # phase C/D pools opened BEFORE phase B emission so the rms/rope work and
        # the q/k transposes can overlap the tail of the qkv matmuls.
        c_cm = tc.tile_pool(name="phC", bufs=2, side="left")
        c_p = c_cm.__enter__()
        c_small_cm = tc.tile_pool(name="phC_small", bufs=2, side="left")
        c_small = c_small_cm.__enter__()
        qkT_cm = tc.tile_pool(name="qkT", bufs=1, side="left")
        qkT_p = qkT_cm.__enter__()
        qT = qkT_p.tile([128, HG, S], BF16)
        kT = qkT_p.tile([128, HG, S], BF16)

        # ======== phase B: qkv projections (natural layout, m-outer) ==============
        psb_cm = tc.tile_pool(name="ps_qkv", bufs=4, space="PSUM")
        psb = psb_cm.__enter__()

        for g in range(6):
            proj, n = divmod(g, 2)
            nat = (qnat, knat, vnat)[proj]
            if g not in wslabs:
                emit_wslab(g)
            wslab = wslabs.pop(g)
            for m in range(NT):
                if g == 0 and m > 0:
                    # build the two aug lhsT rows for this s-tile: a tiny PE
                    # transpose interleaved with the matmul stream (its stats
                    # are ready well before PE reaches this point)
                    ptA = pst.tile([2, 128], BF16, tag="ptA")
                    nc.tensor.transpose(ptA, packs[:, :, m], ident)
                    nc.scalar.copy(out=augT[:, m * 128:(m + 1) * 128], in_=ptA)
                ps = psb.tile([128, 512], F32, tag="psb")
                for k in range(KT):
                    nc.tensor.matmul(ps, xnT[:, k, m * 128:(m + 1) * 128],
                                     wslab[:, k, :],
                                     start=(k == 0), stop=False)
                if g == 0 and m == 0:
                    # m=0's aug rows go AFTER its k-loop: its stats chain is
                    # the longest pole at startup and PE is strictly in-order
                    ptA = pst.tile([2, 128], BF16, tag="ptA")
                    nc.tensor.transpose(ptA, packs[:, :, m], ident)
                    nc.scalar.copy(out=augT[:, m * 128:(m + 1) * 128], in_=ptA)
                # augmented rows: += (1/rstd)*b' + (-mu)*colsum(W')
                nc.tensor.matmul(ps, augT[:, m * 128:(m + 1) * 128],
                                 waug_sb[:, g * 512:(g + 1) * 512],
                                 start=False, stop=True)
                dst = nat[:, m, n * 512:(n + 1) * 512]
                if proj == 2:
                    # v needs the layernorm 1/std (cancels via rmsnorm for q/k)
                    if m % 2 == 0:
                        nc.scalar.activation(out=dst, in_=ps, func=AF.Identity,
                                             scale=rstd_all[:, m:m + 1])
                    else:
                        nc.vector.tensor_scalar(
                            out=dst, in0=ps, scalar1=rstd_all[:, m:m + 1],
                            scalar2=None, op0=mybir.AluOpType.mult)
                elif (g * NT + m) % 2 == 0:
                    nc.scalar.copy(out=dst, in_=ps)
                else:
                    nc.vector.tensor_copy(out=dst, in_=ps)
        if has_qkv_bias:
            for m in range(NT):
                nc.gpsimd.tensor_add(out=qnat[:, m, :], in0=qnat[:, m, :], in1=bq_b)
                nc.gpsimd.tensor_add(out=knat[:, m, :], in0=knat[:, m, :], in1=bk_b)
                nc.gpsimd.tensor_add(out=vnat[:, m, :], in0=vnat[:, m, :], in1=bv_b)

        psb_cm.__exit__(None, None, None)
        pst_cm.__exit__(None, None, None)
        w_cm.__exit__(None, None, None)

        # ======== phase C+D: rmsnorm + rope, then per-head transposes =============
        for m in range(NT):
            qm = qnat[:, m, :]
            km = knat[:, m, :]
            ct = cs_c[:, m, :]
            st = cs_s[:, m, :]
            cb = ct.unsqueeze(1).broadcast_to([128, HG, RD // 2])
            sb_ = st.unsqueeze(1).broadcast_to([128, HG, RD // 2])

            # rms stats (on raw q/k, before norm-w and rope)
            sq = c_p.tile([128, GCOLS], BF16, tag="sqk")
            nc.vector.tensor_mul(out=sq, in0=qm, in1=qm)
            ssq = c_small.tile([128, HG], F32, tag="ssq")
            nc.vector.reduce_sum(out=ssq, in_=sq.rearrange("p (h d) -> p h d", h=HG),
                                 axis=mybir.AxisListType.X)
            rrq = c_small.tile([128, HG], F32, tag="rrq")
            nc.scalar.activation(out=rrq, in_=ssq, func=AF.Sqrt,
                                 bias=eps_t, scale=1.0 / HD)
            nc.vector.reciprocal(out=rrq, in_=rrq)

            sk_ = c_p.tile([128, GCOLS], BF16, tag="sqk")
            nc.vector.tensor_mul(out=sk_, in0=km, in1=km)
            ssk = c_small.tile([128, HG], F32, tag="ssk")
            nc.vector.reduce_sum(out=ssk, in_=sk_.rearrange("p (h d) -> p h d", h=HG),
                                 axis=mybir.AxisListType.X)
            nc.scalar.activation(out=rrk_all[:, m, :], in_=ssk, func=AF.Sqrt,
                                 bias=eps128_t, scale=1.0)
            nc.vector.reciprocal(out=rrk_all[:, m, :], in_=rrk_all[:, m, :])

            if has_norm_w:
                nc.vector.tensor_mul(out=qm.rearrange("p (h d) -> p h d", h=HG),
                                     in0=qm.rearrange("p (h d) -> p h d", h=HG),
                                     in1=wqn_b)
                nc.vector.tensor_mul(out=km.rearrange("p (h d) -> p h d", h=HG),
                                     in0=km.rearrange("p (h d) -> p h d", h=HG),
                                     in1=wkn_b)

            for mm in (qm, km):
                mv_ = mm.rearrange("p (h i two) -> p h i two", h=HG, two=2)
                x0 = mv_[:, :, 0:RD // 2, 0]
                x1 = mv_[:, :, 0:RD // 2, 1]
                t0 = c_small.tile([128, HG, RD // 2], BF16, tag="t0")
                t1 = c_small.tile([128, HG, RD // 2], BF16, tag="t1")
                t2 = c_small.tile([128, HG, RD // 2], BF16, tag="t2")
                t3 = c_small.tile([128, HG, RD // 2], BF16, tag="t3")
                nc.vector.tensor_mul(out=t0, in0=x0, in1=cb)
                nc.vector.tensor_mul(out=t1, in0=x1, in1=sb_)
                nc.vector.tensor_mul(out=t2, in0=x0, in1=sb_)
                nc.vector.tensor_mul(out=t3, in0=x1, in1=cb)
                nc.gpsimd.tensor_sub(out=x0, in0=t0, in1=t1)
                nc.gpsimd.tensor_add(out=x1, in0=t2, in1=t3)

            # apply q rms reciprocal (k's is folded into the exp scale later)
            rrq_b = rrq.unsqueeze(2).broadcast_to([128, HG, HD])
            nc.vector.tensor_mul(out=qm.rearrange("p (h d) -> p h d", h=HG),
                                 in0=qm.rearrange("p (h d) -> p h d", h=HG),
                                 in1=rrq_b)

            # ---- phase D fused: transpose q, k of this s-tile -> [hd, head, s]
            # on the DMA xbar (PE and ACT/DVE stay free for qkv/attention
            # work).  Issued from the ACT queue — the SP queue is busy waiting
            # out weight-slab buffer reuse at this point.
            nc.scalar.dma_start_transpose(
                out=qT[:, :, m * 128:(m + 1) * 128], in_=qnat[:, m, :])
            nc.scalar.dma_start_transpose(
                out=kT[:, :, m * 128:(m + 1) * 128], in_=knat[:, m, :])

        # wo slab load deferred to here: DMA engines are busy with x/xnT/slab
        # traffic early on, and wo isn't consumed until the out projection
        nc.sync.dma_start(out=wo_sb, in_=wo.rearrange("(kb p) d -> p kb d", p=128))

        natqk_cm.__exit__(None, None, None)

        # ======== phase E: attention per head ======================================
        oT_cm = tc.tile_pool(name="oT", bufs=1, side="left")
        oT_p = oT_cm.__enter__()
        oT = oT_p.tile([128, HG, S], BF16)
        at_cm = tc.tile_pool(name="attn", bufs=6, side="left")
        at_p = at_cm.__enter__()
        rs_cm = tc.tile_pool(name="rsb", bufs=3, side="left")
        rs_p = rs_cm.__enter__()
        pssc_cm = tc.tile_pool(name="ps_sc", bufs=2, space="PSUM")
        pssc = pssc_cm.__enter__()
        pso_cm = tc.tile_pool(name="ps_o", bufs=2, space="PSUM")
        pso = pso_cm.__enter__()

        for h in range(HG):
            o_ps = pso.tile([128, S], F32, tag="o_ps")
            acc = rs_p.tile([128, S], BF16, tag="acc")
            for m in range(NT):
                sc = pssc.tile([128, S], F32, tag="sc")
                lhs_k = kT[:, h, m * 128:(m + 1) * 128]
                nc.tensor.matmul(sc[:, 0:512], lhs_k, qT[:, h, 0:512],
                                 start=True, stop=True)
                nc.tensor.matmul(sc[:, 512:1024], lhs_k, qT[:, h, 512:1024],
                                 start=True, stop=True)
                at = at_p.tile([128, S], BF16, tag="at", name="at")
                nc.scalar.activation(out=at, in_=sc, func=AF.Exp,
                                     scale=rrk_all[:, m, h:h + 1])
                # accumulate exp tiles on DVE (sums over the m-tiles)
                if m == 0:
                    nc.vector.tensor_copy(out=acc, in_=at)
                else:
                    nc.vector.tensor_add(out=acc, in0=acc, in1=at)
                first, last = (m == 0), (m == NT - 1)
                v_mh = vnat[:, m, h * 128:(h + 1) * 128]
                nc.tensor.matmul(o_ps[:, 0:512], v_mh, at[:, 0:512],
                                 start=first, stop=last)
                nc.tensor.matmul(o_ps[:, 512:1024], v_mh, at[:, 512:1024],
                                 start=first, stop=last)
            # sum over the sk partitions -> broadcast row, then normalize.
            # Done in column halves: halves the latency of the
            # all_reduce->reciprocal->mul chain, which otherwise stalls the
            # out-projection's first psum group waiting on the last head.
            sums_b = rs_p.tile([128, S], F32, tag="sums_b")
            for hh in range(2):
                cl = slice(hh * 512, (hh + 1) * 512)
                nc.gpsimd.partition_all_reduce(sums_b[:, cl], acc[:, cl], 128,
                                               bass_isa.ReduceOp.add)
                nc.vector.reciprocal(out=sums_b[:, cl], in_=sums_b[:, cl])
                nc.vector.tensor_mul(out=oT[:, h, cl], in0=o_ps[:, cl],
                                     in1=sums_b[:, cl])

        pso_cm.__exit__(None, None, None)
        pssc_cm.__exit__(None, None, None)
        rs_cm.__exit__(None, None, None)
        at_cm.__exit__(None, None, None)
        v_cm.__exit__(None, None, None)

        # ======== phase F: out projection (transposed out) =========================
        f_cm = tc.tile_pool(name="phF", bufs=2, side="left")
        f_p = f_cm.__enter__()
        psf_cm = tc.tile_pool(name="ps_out", bufs=2, space="PSUM")
        psf = psf_cm.__enter__()
        for mb in range(KT // 2):
            stage = f_p.tile([128, 2, S], F32, tag="stage")
            for mm in range(2):
                m = mb * 2 + mm
                po = psf.tile([128, S], F32, tag="po")
                for kb in range(HG):
                    first, last = (kb == 0), (kb == HG - 1)
                    nc.tensor.matmul(po[:, 0:512],
                                     wo_sb[:, kb, m * 128:(m + 1) * 128],
                                     oT[:, kb, 0:512], start=first, stop=last)
                    nc.tensor.matmul(po[:, 512:1024],
                                     wo_sb[:, kb, m * 128:(m + 1) * 128],
                                     oT[:, kb, 512:1024], start=first, stop=last)
                nc.scalar.activation(out=stage[:, mm, :], in_=po, func=AF.Identity,
                                     bias=vb_sb[:, m:m + 1], scale=gate_sb[:, m:m + 1])
            nc.sync.dma_start(
                out=out_t[mb * 256:(mb + 1) * 256, :].rearrange(
                    "(mm p) s -> p mm s", p=128),
                in_=stage)
        psf_cm.__exit__(None, None, None)
        f_cm.__exit__(None, None, None)
        oT_cm.__exit__(None, None, None)
        qkT_cm.__exit__(None, None, None)
        c_small_cm.__exit__(None, None, None)
        c_cm.__exit__(None, None, None)
        avec_cm.__exit__(None, None, None)
        xnT_cm.__exit__(None, None, None)
        wop_cm.__exit__(None, None, None)
        misc_cm.__exit__(None, None, None)

    nc.compile()
    return nc


_NC_CACHE = {}


def _get_nc(has_qkv_bias, has_norm_w):
    key = (has_qkv_bias, has_norm_w)
    if key not in _NC_CACHE:
        _NC_CACHE[key] = build_nc(*key)
    return _NC_CACHE[key]


def prep_in_maps(x, mod, cos, sin, qkv_w, qkv_b, mod_w, mod_b, out_w, out_b,
                 norm_q_w, norm_k_w):
    """Host-side sharding. Returns (in_maps, flags, x_np)."""
    x = np.asarray(x, dtype=np.float32)
    m3 = np.asarray(mod, np.float32) @ np.asarray(mod_w, np.float32) \
        + np.asarray(mod_b, np.float32)
    bias, scale, gatef = np.split(m3, 3, axis=-1)          # [B, D] each
    scale1p = (1.0 + scale).astype(np.float32)
    vbf = (np.asarray(out_b, np.float32)[None, :] * gatef).astype(np.float32)

    qkv_b = np.asarray(qkv_b, np.float32)
    has_qkv_bias = bool(np.any(qkv_b != 0.0))
    has_norm_w = not (np.allclose(norm_q_w, 1.0) and np.allclose(norm_k_w, 1.0))

    import ml_dtypes
    bf16 = ml_dtypes.bfloat16
    cosc = np.ascontiguousarray(np.asarray(cos, np.float32).astype(bf16))
    sinc = np.ascontiguousarray(np.asarray(sin, np.float32).astype(bf16))
    qkv_w = np.asarray(qkv_w, np.float32)
    out_w = np.asarray(out_w, np.float32).astype(bf16)
    # b' = bias @ W (the modulation bias term, unscaled W): [B, 3D]
    bprime = bias @ qkv_w

    in_maps = []
    for c in range(N_CORES):
        b, g = divmod(c, 2)
        lo = g * GCOLS
        # fold (1+scale) into the weight rows for this batch
        wmod = qkv_w * scale1p[b][:, None]          # [D, 3D] fp32
        cols = [wmod[:, p * 2048 + lo:p * 2048 + lo + GCOLS] for p in range(3)]
        bcols = [bprime[b, p * 2048 + lo:p * 2048 + lo + GCOLS] for p in range(3)]
        w1 = np.concatenate([cw.sum(axis=0) for cw in cols])       # [3*GCOLS]
        b2 = np.concatenate(bcols)                                 # [3*GCOLS]
        xbf = np.ascontiguousarray(x[b].astype(bf16))
        im = {
            "x": xbf,
            "xt_src": xbf.copy(),
            "cos": cosc, "sin": sinc,
            "wq": np.ascontiguousarray(cols[0].astype(bf16)),
            "wk": np.ascontiguousarray(cols[1].astype(bf16)),
            "wv": np.ascontiguousarray(cols[2].astype(bf16)),
            "wo": np.ascontiguousarray(out_w[lo:lo + GCOLS, :]),
            "waug": np.ascontiguousarray(
                np.stack([b2, w1]).astype(bf16)),                  # [2, 3*GCOLS]
            "gate": np.ascontiguousarray(gatef[b].reshape(KT, 128).T),
            "vb": np.ascontiguousarray(
                (vbf[b] if g == 0 else np.zeros_like(vbf[b])).reshape(KT, 128).T),
        }
        if has_qkv_bias:
            im["bq"] = np.ascontiguousarray(qkv_b[lo:lo + GCOLS].astype(bf16))
            im["bk"] = np.ascontiguousarray(
                qkv_b[2048 + lo:2048 + lo + GCOLS].astype(bf16))
            im["bv"] = np.ascontiguousarray(
                qkv_b[4096 + lo:4096 + lo + GCOLS].astype(bf16))
        if has_norm_w:
            im["wqn"] = np.ascontiguousarray(
                np.asarray(norm_q_w, np.float32).astype(bf16))
            im["wkn"] = np.ascontiguousarray(
                np.asarray(norm_k_w, np.float32).astype(bf16))
        in_maps.append(im)
    return in_maps, (has_qkv_bias, has_norm_w), x


def gather(results, x):
    B = x.shape[0]
    outs = []
    for b in range(B):
        p = results[2 * b]["out_t"] + results[2 * b + 1]["out_t"]   # [D, S]
        outs.append(p.T + x[b])
    return np.stack(outs).astype(np.float32)


def kernel(**inputs) -> np.ndarray:
    in_maps, flags, x = prep_in_maps(**inputs)
    nc = _get_nc(*flags)
    res = run_bass_kernel_spmd(nc, in_maps, core_ids=list(range(N_CORES)))
    return gather(res.results, x)


if __name__ == "__main__":
    import time
    t0 = time.time()
    nc = build_nc(False, False)
    print("build+compile ok in", time.time() - t0, "s")
